# revision 1
# baseline (speedup 1.0000x reference)
"""Trainium2 Bass kernel for nn_MiddleBlock (Chebyshev graph conv + pseudo-conv).

Reference computation (B=2, N=196608, FIN=64, K=4, FOUT=128, NB=8):
  T0 = x; T1 = L x; T_k = 2 L T_{k-1} - T_{k-2}   with
  (L y)[i] = sum_k edge_w[i,k] * y[nbr[i,k]] + diag[i] * y[i]
  h = relu(concat(T0..T3) @ cheb_w + cheb_b); h = LN(h)
  h = relu(h.reshape(B, N/4, 4*FOUT) @ pc_w + pc_b); h = LN(h)

Distribution over 8 NeuronCores: both batches are fused into the column dim
(col = b*64 + f, 512B rows) and the pixel axis N is sharded 8 ways.  Each
apply_L gathers neighbor rows from a full (replicated) copy of the previous
term via indirect DMA; after T1 and T2 an AllGather rebuilds the full table.
The dense phase (cheb matmul, LN, pseudo-conv, LN) is fully local per shard.
"""

import dataclasses
import functools

import numpy as np

B, FIN, K, FOUT, NB = 2, 64, 4, 128, 8
C = B * FIN  # fused column dim = 128
EPS = 1e-6
P = 128


@dataclasses.dataclass(frozen=True)
class Cfg:
    n: int = 196608
    ncores: int = 8
    tpg: int = 8  # target tiles per gather supertile (apply phase)
    mts: int = 8  # target tiles per matmul supertile (must be mult of 4)
    l1aff: bool = False  # apply gamma1/beta1
    l2aff: bool = False
    dbg: bool = False  # add debug outputs for intermediates
    noop: bool = False  # timing baseline: same I/O, no compute

    @property
    def ns(self):
        return self.n // self.ncores

    @property
    def nsa(self):  # apply supertiles per core
        return self.ns // (self.tpg * P)

    @property
    def nsm(self):  # matmul supertiles per core
        return self.ns // (self.mts * P)


FULL = Cfg()


# ------------------------------------------------------------------ host prep


def host_prep(inputs: dict, cfg: Cfg) -> tuple[list[dict], dict]:
    """Build per-core input maps for the SPMD kernel."""
    x = np.asarray(inputs["x"], np.float32)
    nbr = np.asarray(inputs["nbr_idx"], np.int32)
    ew = np.asarray(inputs["edge_w"], np.float32)
    diag = np.asarray(inputs["diag"], np.float32)
    cw = np.asarray(inputs["cheb_w"], np.float32)
    cb = np.asarray(inputs["cheb_b"], np.float32)
    pw = np.asarray(inputs["pc_w"], np.float32)
    pb = np.asarray(inputs["pc_b"], np.float32)
    g1 = np.asarray(inputs["gamma1"], np.float32)
    b1 = np.asarray(inputs["beta1"], np.float32)
    g2 = np.asarray(inputs["gamma2"], np.float32)
    b2 = np.asarray(inputs["beta2"], np.float32)

    nc_, nsa, tpg = cfg.ncores, cfg.nsa, cfg.tpg
    x2 = np.ascontiguousarray(np.concatenate([x[0], x[1]], axis=1))  # [N, C]

    def shard_sup(arr, w):
        # [N, w] -> [ncores, 128, nsa*tpg*w]; col order: (supertile, tile, w)
        a = arr.reshape(nc_, nsa, tpg, P, w)
        a = a.transpose(0, 3, 1, 2, 4)
        return np.ascontiguousarray(a.reshape(nc_, P, nsa * tpg * w))

    idxs = shard_sup(nbr, NB)
    ew1s = shard_sup(ew, NB)
    ew2s = shard_sup(2.0 * ew, NB)
    dg1s = shard_sup(diag[:, None], 1)
    dg2s = shard_sup(2.0 * diag[:, None], 1)
    xs = np.ascontiguousarray(x2.reshape(nc_, cfg.ns, C))

    cwT = np.zeros((K, 2, C, FOUT), np.float32)
    for k in range(K):
        for b in range(2):
            cwT[k, b, b * FIN : (b + 1) * FIN, :] = cw[k * FIN : (k + 1) * FIN, :]
    pwT = np.ascontiguousarray(pw.reshape(4, FOUT, FOUT))

    shared = {
        "x2": x2,
        "cwT": cwT,
        "pwT": pwT,
        "cb": np.ascontiguousarray(cb.reshape(FOUT, 1)),
        "pb": np.ascontiguousarray(pb.reshape(FOUT, 1)),
    }
    if cfg.l1aff:
        shared["g1r"] = np.ascontiguousarray(np.broadcast_to(g1, (P, FOUT)))
        shared["b1r"] = np.ascontiguousarray(np.broadcast_to(b1, (P, FOUT)))
    if cfg.l2aff:
        shared["g2r"] = np.ascontiguousarray(np.broadcast_to(g2, (P, FOUT)))
        shared["b2r"] = np.ascontiguousarray(np.broadcast_to(b2, (P, FOUT)))

    in_maps = []
    for c in range(nc_):
        m = dict(shared)
        m["xs"] = xs[c]
        m["idxs"] = idxs[c]
        m["ew1s"] = ew1s[c]
        m["ew2s"] = ew2s[c]
        m["dg1s"] = dg1s[c]
        m["dg2s"] = dg2s[c]
        in_maps.append(m)
    return in_maps, shared


# ------------------------------------------------------------------ builder


def build_nc(cfg: Cfg):
    import concourse.bacc as bacc
    import concourse.bass as bass
    import concourse.mybir as mybir
    import concourse.tile as tile
    from concourse.masks import make_identity

    dt = mybir.dt
    f32 = dt.float32
    i32 = dt.int32
    Alu = mybir.AluOpType
    Act = mybir.ActivationFunctionType
    Ax = mybir.AxisListType

    NS, nsa, nsm, tpg, mts = cfg.ns, cfg.nsa, cfg.nsm, cfg.tpg, cfg.mts
    TW = tpg * NB

    nc = bacc.Bacc(
        "TRN2",
        target_bir_lowering=False,
        debug=False,
        enable_asserts=False,
        num_devices=cfg.ncores,
    )

    x2 = nc.dram_tensor("x2", [cfg.n, C], f32, kind="ExternalInput")
    xs = nc.dram_tensor("xs", [NS, C], f32, kind="ExternalInput")
    idxs_d = nc.dram_tensor("idxs", [P, nsa * TW], i32, kind="ExternalInput")
    ew1_d = nc.dram_tensor("ew1s", [P, nsa * TW], f32, kind="ExternalInput")
    ew2_d = nc.dram_tensor("ew2s", [P, nsa * TW], f32, kind="ExternalInput")
    dg1_d = nc.dram_tensor("dg1s", [P, nsa * tpg], f32, kind="ExternalInput")
    dg2_d = nc.dram_tensor("dg2s", [P, nsa * tpg], f32, kind="ExternalInput")
    cwT_d = nc.dram_tensor("cwT", [K, 2, C, FOUT], f32, kind="ExternalInput")
    pwT_d = nc.dram_tensor("pwT", [4, FOUT, FOUT], f32, kind="ExternalInput")
    cb_d = nc.dram_tensor("cb", [FOUT, 1], f32, kind="ExternalInput")
    pb_d = nc.dram_tensor("pb", [FOUT, 1], f32, kind="ExternalInput")
    aff_d = {}
    if cfg.l1aff:
        aff_d["g1r"] = nc.dram_tensor("g1r", [P, FOUT], f32, kind="ExternalInput")
        aff_d["b1r"] = nc.dram_tensor("b1r", [P, FOUT], f32, kind="ExternalInput")
    if cfg.l2aff:
        aff_d["g2r"] = nc.dram_tensor("g2r", [P, FOUT], f32, kind="ExternalInput")
        aff_d["b2r"] = nc.dram_tensor("b2r", [P, FOUT], f32, kind="ExternalInput")
    out_d = nc.dram_tensor("out", [B, NS // 4, FOUT], f32, kind="ExternalOutput")
    dbg_d = {}
    if cfg.dbg:
        for nm, shp in [
            ("d_t1s", [NS, C]),
            ("d_t1f", [cfg.n, C]),
            ("d_t2s", [NS, C]),
            ("d_t3s", [NS, C]),
        ]:
            dbg_d[nm] = nc.dram_tensor(nm, shp, f32, kind="ExternalOutput")

    if cfg.noop:
        # Timing baseline: identical external I/O, minimal device work.
        with tile.TileContext(nc) as tc:
            with tc.tile_pool(name="sb", bufs=1) as sb:
                z = sb.tile([P, NS // 4], f32, name="z")
                nc.vector.memset(z[:], 0.0)
                for b in range(B):
                    nc.sync.dma_start(
                        out=out_d.ap()[b].rearrange("(t p) c -> p t c", p=P),
                        in_=z[:].rearrange("p (t c) -> p t c", c=FOUT),
                    )
        nc.compile()
        return nc

    with tile.TileContext(nc) as tc:
        with (
            tc.tile_pool(name="const", bufs=1) as pc_,
            tc.tile_pool(name="dram", bufs=1, space="DRAM") as pdram,
        ):
            # persistent DRAM intermediates
            t1s = pdram.tile([NS, C], f32, name="t1s")
            t2s = pdram.tile([NS, C], f32, name="t2s")
            t3s = pdram.tile([NS, C], f32, name="t3s")
            t1f = pdram.tile([cfg.n, C], f32, name="t1f", addr_space="Shared")
            t2f = pdram.tile([cfg.n, C], f32, name="t2f", addr_space="Shared")

            # resident constants
            ident = pc_.tile([P, P], f32, name="ident")
            make_identity(nc, ident[:])
            idx_all = pc_.tile([P, nsa * TW], i32, name="idx_all")
            nc.sync.dma_start(out=idx_all[:], in_=idxs_d[:, :])
            ew1_all = pc_.tile([P, nsa * TW], f32, name="ew1_all")
            nc.sync.dma_start(out=ew1_all[:], in_=ew1_d[:, :])
            ew2_all = pc_.tile([P, nsa * TW], f32, name="ew2_all")
            nc.sync.dma_start(out=ew2_all[:], in_=ew2_d[:, :])
            dg1_all = pc_.tile([P, nsa * tpg], f32, name="dg1_all")
            nc.sync.dma_start(out=dg1_all[:], in_=dg1_d[:, :])
            dg2_all = pc_.tile([P, nsa * tpg], f32, name="dg2_all")
            nc.sync.dma_start(out=dg2_all[:], in_=dg2_d[:, :])
            cw_sb = []
            for k in range(K):
                row = []
                for b in range(2):
                    t = pc_.tile([C, FOUT], f32, name=f"cw_{k}_{b}")
                    nc.sync.dma_start(out=t[:], in_=cwT_d[k, b])
                    row.append(t)
                cw_sb.append(row)
            pw_sb = []
            for r in range(4):
                t = pc_.tile([FOUT, FOUT], f32, name=f"pw_{r}")
                nc.sync.dma_start(out=t[:], in_=pwT_d[r])
                pw_sb.append(t)
            cb_sb = pc_.tile([FOUT, 1], f32, name="cb_sb")
            nc.sync.dma_start(out=cb_sb[:], in_=cb_d[:, :])
            pb_sb = pc_.tile([FOUT, 1], f32, name="pb_sb")
            nc.sync.dma_start(out=pb_sb[:], in_=pb_d[:, :])
            aff_sb = {}
            for nm, d in aff_d.items():
                t = pc_.tile([P, FOUT], f32, name=f"{nm}_sb")
                nc.sync.dma_start(out=t[:], in_=d[:, :])
                aff_sb[nm] = t

            # ---------------- apply phase ----------------
            def emit_apply(src_full, own_cur, prev, ew_all, dg_all, dst, pools):
                p_g, p_io = pools
                for s in range(nsa):
                    rows = slice(s * tpg * P, (s + 1) * tpg * P)
                    own = p_io.tile([P, tpg * C], f32, name="own", tag="own")
                    nc.sync.dma_start(
                        out=own[:].rearrange("p (t c) -> p t c", t=tpg),
                        in_=own_cur[rows, :].rearrange("(t p) c -> p t c", p=P),
                    )
                    if prev is not None:
                        prv = p_io.tile([P, tpg * C], f32, name="prv", tag="prv")
                        nc.sync.dma_start(
                            out=prv[:].rearrange("p (t c) -> p t c", t=tpg),
                            in_=prev[rows, :].rearrange("(t p) c -> p t c", p=P),
                        )
                    G = p_g.tile([P, TW * C], f32, name="G", tag="G")
                    for j in range(TW):
                        # one indirect DMA per 128 gathered rows (HW consumes
                        # exactly one offset per partition; batching the
                        # offset AP to [128, TW] passes MultiCoreSim but
                        # returns wrong data on real HW — verified 2026-08-09)
                        nc.gpsimd.indirect_dma_start(
                            out=G[:, j * C : (j + 1) * C],
                            out_offset=None,
                            in_=src_full,
                            in_offset=bass.IndirectOffsetOnAxis(
                                ap=idx_all[:, s * TW + j : s * TW + j + 1], axis=0
                            ),
                        )
                    tn = p_io.tile([P, tpg * C], f32, name="tn", tag="tn")
                    # G[:, (t k c)] *= ew[t, k], weight broadcast over c
                    Gv = G[:].rearrange("p (j c) -> p j c", j=TW)
                    nc.vector.tensor_tensor(
                        out=Gv,
                        in0=Gv,
                        in1=ew_all[:, s * TW : (s + 1) * TW].to_broadcast(
                            [P, TW, C]
                        ),
                        op=Alu.mult,
                    )
                    # tn[p, t, c] = sum_k G[p, t, k, c]  (innermost-axis reduce)
                    nc.vector.tensor_reduce(
                        tn[:],
                        G[:].rearrange("p (t k c) -> p t c k", t=tpg, k=NB),
                        axis=Ax.X,
                        op=Alu.add,
                    )
                    # own-term: od = own * diag (broadcast over c), then
                    # tn += od [- prev]
                    od = p_io.tile([P, tpg * C], f32, name="od", tag="od")
                    nc.vector.tensor_tensor(
                        out=od[:].rearrange("p (t c) -> p t c", t=tpg),
                        in0=own[:].rearrange("p (t c) -> p t c", t=tpg),
                        in1=dg_all[:, s * tpg : (s + 1) * tpg].to_broadcast(
                            [P, tpg, C]
                        ),
                        op=Alu.mult,
                    )
                    nc.vector.tensor_tensor(
                        out=tn[:], in0=tn[:], in1=od[:], op=Alu.add
                    )
                    if prev is not None:
                        nc.vector.tensor_tensor(
                            out=tn[:], in0=tn[:], in1=prv[:], op=Alu.subtract
                        )
                    nc.sync.dma_start(
                        out=dst[rows, :].rearrange("(t p) c -> p t c", p=P),
                        in_=tn[:].rearrange("p (t c) -> p t c", t=tpg),
                    )

            rg = [list(range(cfg.ncores))]
            with (
                tc.tile_pool(name="p_g", bufs=3) as p_g,
                tc.tile_pool(name="p_io", bufs=4) as p_io,
            ):
                pools = (p_g, p_io)
                emit_apply(x2[:, :], xs[:, :], None, ew1_all, dg1_all, t1s[:, :], pools)
                nc.gpsimd.collective_compute(
                    "AllGather",
                    Alu.bypass,
                    replica_groups=rg,
                    ins=[t1s.opt()],
                    outs=[t1f.opt()],
                )
                emit_apply(
                    t1f[:, :], t1s[:, :], xs[:, :], ew2_all, dg2_all, t2s[:, :], pools
                )
                nc.gpsimd.collective_compute(
                    "AllGather",
                    Alu.bypass,
                    replica_groups=rg,
                    ins=[t2s.opt()],
                    outs=[t2f.opt()],
                )
                emit_apply(
                    t2f[:, :], t2s[:, :], t1s[:, :], ew2_all, dg2_all, t3s[:, :], pools
                )
                if cfg.dbg:
                    nc.sync.dma_start(out=dbg_d["d_t1s"][:, :], in_=t1s[:, :])
                    nc.sync.dma_start(out=dbg_d["d_t1f"][:, :], in_=t1f[:, :])
                    nc.sync.dma_start(out=dbg_d["d_t2s"][:, :], in_=t2s[:, :])
                    nc.sync.dma_start(out=dbg_d["d_t3s"][:, :], in_=t3s[:, :])

            # ---------------- dense phase ----------------
            def emit_ln(psr, out_sbuf, gb_tuple, p_small, p_scr):
                # psr: [128, FOUT] row-major (PSUM); out_sbuf: [128, FOUT] SBUF
                ssum = p_small.tile([P, 1], f32, name="ssum", tag="ssum")
                nc.vector.tensor_reduce(ssum[:], psr, axis=Ax.X, op=Alu.add)
                negmu = p_small.tile([P, 1], f32, name="negmu", tag="negmu")
                nc.vector.tensor_scalar_mul(negmu[:], ssum[:], -1.0 / FOUT)
                sq = p_scr.tile([P, FOUT], f32, name="sq", tag="sq")
                vp0 = p_small.tile([P, 1], f32, name="vp0", tag="vp0")
                nc.scalar.activation(
                    sq[:],
                    psr,
                    Act.Square,
                    bias=negmu[:, :1],
                    scale=1.0,
                    accum_out=vp0[:],
                )
                vpe = p_small.tile([P, 1], f32, name="vpe", tag="vpe")
                nc.vector.tensor_scalar(
                    vpe[:], vp0[:], 1.0 / FOUT, EPS, op0=Alu.mult, op1=Alu.add
                )
                sig = p_small.tile([P, 1], f32, name="sig", tag="sig")
                nc.scalar.sqrt(sig[:], vpe[:])
                rsig = p_small.tile([P, 1], f32, name="rsig", tag="rsig")
                nc.vector.reciprocal(rsig[:], sig[:])
                nmr = p_small.tile([P, 1], f32, name="nmr", tag="nmr")
                nc.vector.tensor_tensor(
                    out=nmr[:], in0=negmu[:], in1=rsig[:], op=Alu.mult
                )
                if gb_tuple is None:
                    nc.scalar.activation(
                        out_sbuf, psr, Act.Identity, bias=nmr[:, :1],
                        scale=rsig[:, :1],
                    )
                else:
                    gtmp = p_scr.tile([P, FOUT], f32, name="gtmp", tag="gtmp")
                    nc.scalar.activation(
                        gtmp[:], psr, Act.Identity, bias=nmr[:, :1],
                        scale=rsig[:, :1],
                    )
                    gb, bb = gb_tuple
                    nc.vector.tensor_tensor(
                        out=gtmp[:], in0=gtmp[:], in1=gb[:], op=Alu.mult
                    )
                    nc.vector.tensor_tensor(
                        out=out_sbuf, in0=gtmp[:], in1=bb[:], op=Alu.add
                    )

            ln1_gb = (aff_sb["g1r"], aff_sb["b1r"]) if cfg.l1aff else None
            ln2_gb = (aff_sb["g2r"], aff_sb["b2r"]) if cfg.l2aff else None

            with (
                tc.tile_pool(name="p_ld", bufs=4) as p_ld,
                tc.tile_pool(name="p_tt", bufs=2) as p_tt,
                tc.tile_pool(name="p_h1", bufs=2) as p_h1,
                tc.tile_pool(name="p_cs", bufs=4) as p_cs,
                tc.tile_pool(name="p_small", bufs=8) as p_small,
                tc.tile_pool(name="p_psT", bufs=4, space="PSUM") as p_psT,
                tc.tile_pool(name="p_psM", bufs=4, space="PSUM") as p_psM,
            ):
                srcs = [xs[:, :], t1s[:, :], t2s[:, :], t3s[:, :]]
                for m in range(nsm):
                    # stage A: load + transpose T_k tiles -> TkT [C, mts*128]
                    TkT = []
                    for k in range(K):
                        tkt = p_tt.tile(
                            [P, mts * P], f32, name=f"TkT{k}", tag=f"TkT{k}"
                        )
                        for t0 in range(0, mts, 4):
                            pst = p_psT.tile(
                                [P, 4 * P], f32, name="pst", tag="pst", space="PSUM"
                            )
                            r0 = m * mts * P + t0 * P
                            ld4 = p_ld.tile([P, 4 * P], f32, name="ld", tag="ld")
                            nc.sync.dma_start(
                                out=ld4[:].rearrange("p (t c) -> p t c", t=4),
                                in_=srcs[k][r0 : r0 + 4 * P, :].rearrange(
                                    "(t p) c -> p t c", p=P
                                ),
                            )
                            for i in range(4):
                                nc.tensor.transpose(
                                    pst[:, i * P : (i + 1) * P],
                                    ld4[:, i * P : (i + 1) * P],
                                    ident[:],
                                )
                            nc.vector.tensor_copy(
                                out=tkt[:, t0 * P : (t0 + 4) * P], in_=pst[:]
                            )
                        TkT.append(tkt)
                    # stage B: cheb matmul -> hps[b][half] [FOUT, 512] psum
                    hps = [[None, None], [None, None]]
                    for b in range(2):
                        for h in range(mts * P // 512):
                            ps = p_psM.tile(
                                [P, 512], f32, name="hps", tag="hps", space="PSUM"
                            )
                            hps[b][h] = ps
                            for k in range(K):
                                nc.tensor.matmul(
                                    ps[:],
                                    cw_sb[k][b][:],
                                    TkT[k][:, h * 512 : (h + 1) * 512],
                                    start=(k == 0),
                                    stop=(k == K - 1),
                                )
                    # stage C: relu+LN1 per [128] target tile; build h1T [C, mts*128]
                    h1T = []
                    for b in range(2):
                        h1t = p_h1.tile(
                            [P, mts * P], f32, name=f"h1T{b}", tag=f"h1T{b}"
                        )
                        hTs_l = []
                        for h in range(mts * P // 512):
                            hTs = p_cs.tile([P, 512], f32, name="hTs", tag="hTs")
                            nc.scalar.activation(
                                hTs[:],
                                hps[b][h][:],
                                Act.Relu,
                                bias=cb_sb[:, :1],
                                scale=1.0,
                            )
                            hTs_l.append(hTs)
                        for t0 in range(0, mts, 4):
                            psb4 = p_psT.tile(
                                [P, 4 * P], f32, name="psb4", tag="pst",
                                space="PSUM",
                            )
                            for i in range(4):
                                t = t0 + i
                                h, off = divmod(t * P, 512)
                                psr = p_psT.tile(
                                    [P, P], f32, name="psr", tag="pst",
                                    space="PSUM",
                                )
                                nc.tensor.transpose(
                                    psr[:], hTs_l[h][:, off : off + P], ident[:]
                                )
                                h1row = p_cs.tile(
                                    [P, P], f32, name="h1row", tag="h1row"
                                )
                                emit_ln(psr[:], h1row[:], ln1_gb, p_small, p_cs)
                                nc.tensor.transpose(
                                    psb4[:, i * P : (i + 1) * P], h1row[:],
                                    ident[:],
                                )
                            nc.vector.tensor_copy(
                                out=h1t[:, t0 * P : (t0 + 4) * P], in_=psb4[:]
                            )
                        h1T.append(h1t)
                    # stage D: pseudo-conv matmul -> h2ps[b] [FOUT, mts*32]
                    noc = mts * P // 4  # out targets per supertile
                    h2ps = []
                    for b in range(2):
                        ps = p_psM.tile(
                            [P, noc], f32, name="h2ps", tag="hps", space="PSUM"
                        )
                        rview = h1T[b][:].rearrange("p (i r) -> p r i", r=4)
                        for r in range(4):
                            nc.tensor.matmul(
                                ps[:],
                                pw_sb[r][:],
                                rview[:, r, :],
                                start=(r == 0),
                                stop=(r == 3),
                            )
                        h2ps.append(ps)
                    # stage E: relu+LN2, write out
                    for b in range(2):
                        h2Ts = p_cs.tile([P, noc], f32, name="h2Ts", tag="hTs")
                        nc.scalar.activation(
                            h2Ts[:],
                            h2ps[b][:],
                            Act.Relu,
                            bias=pb_sb[:, :1],
                            scale=1.0,
                        )
                        for j in range(noc // P):
                            psr2 = p_psT.tile(
                                [P, P], f32, name="psr2", tag="pst", space="PSUM"
                            )
                            nc.tensor.transpose(
                                psr2[:], h2Ts[:, j * P : (j + 1) * P], ident[:]
                            )
                            orow = p_cs.tile([P, P], f32, name="orow", tag="orow")
                            emit_ln(psr2[:], orow[:], ln2_gb, p_small, p_cs)
                            o0 = m * (mts * P // 4) + j * P
                            nc.sync.dma_start(
                                out=out_d.ap()[b, o0 : o0 + P, :], in_=orow[:]
                            )

    nc.compile()
    return nc


# ------------------------------------------------------------------ entry


@functools.lru_cache(maxsize=4)
def _compiled(cfg: Cfg):
    return build_nc(cfg)


def kernel(**inputs) -> np.ndarray:
    from concourse.bass_utils import run_bass_kernel_spmd

    n = inputs["x"].shape[1]
    cfg = dataclasses.replace(
        FULL,
        n=n,
        l1aff=not (
            np.all(np.asarray(inputs["gamma1"]) == 1.0)
            and np.all(np.asarray(inputs["beta1"]) == 0.0)
        ),
        l2aff=not (
            np.all(np.asarray(inputs["gamma2"]) == 1.0)
            and np.all(np.asarray(inputs["beta2"]) == 0.0)
        ),
    )
    nc = _compiled(cfg)
    in_maps, _ = host_prep(inputs, cfg)
    res = run_bass_kernel_spmd(nc, in_maps, list(range(cfg.ncores)))
    out = np.concatenate([res.results[i]["out"] for i in range(cfg.ncores)], axis=1)
    return np.ascontiguousarray(out)



# revision 8
# speedup vs baseline: 224.8902x; 224.8902x over previous
"""Trainium2 Bass kernel for nn_MiddleBlock (Chebyshev graph conv + pseudo-conv).

Reference computation (B=2, N=196608, FIN=64, K=4, FOUT=128, NB=8):
  T0 = x; T1 = L x; T_k = 2 L T_{k-1} - T_{k-2}   with
  (L y)[i] = sum_k edge_w[i,k] * y[nbr[i,k]] + diag[i] * y[i]
  h = relu(concat(T0..T3) @ cheb_w + cheb_b); h = LN(h)
  h = relu(h.reshape(B, N/4, 4*FOUT) @ pc_w + pc_b); h = LN(h)

Distribution over 8 NeuronCores: both batches are fused into the column dim
(col = b*64 + f, 512B rows) and the pixel axis N is sharded 8 ways.  Each
core receives only its own shard of x (no host-side replication); the full
gather table for each term is built on-device with an AllGather.  Each
apply_L gathers neighbor rows from the replicated table via indirect DMA;
the dense phase (cheb matmul, LN, pseudo-conv, LN) is fully local per shard.
"""

import dataclasses
import functools

import numpy as np

B, FIN, K, FOUT, NB = 2, 64, 4, 128, 8
C = B * FIN  # fused column dim = 128
EPS = 1e-6
P = 128


@dataclasses.dataclass(frozen=True)
class Cfg:
    n: int = 196608
    ncores: int = 8
    tpg: int = 8  # target tiles per gather supertile (apply phase)
    mts: int = 8  # target tiles per matmul supertile (must be mult of 4)
    l1aff: bool = False  # apply gamma1/beta1
    l2aff: bool = False
    noop: bool = False  # timing baseline: same I/O, no compute
    phase: str = "all"  # all | apply | apply1 | ag | dense  (perf bisection)

    @property
    def ns(self):
        return self.n // self.ncores

    @property
    def nsa(self):  # apply supertiles per core
        return self.ns // (self.tpg * P)

    @property
    def nsm(self):  # matmul supertiles per core
        return self.ns // (self.mts * P)


FULL = Cfg()


# ------------------------------------------------------------------ host prep


def host_prep(inputs: dict, cfg: Cfg) -> list[dict]:
    """Build per-core input maps for the SPMD kernel."""
    x = np.asarray(inputs["x"], np.float32)
    nbr = np.asarray(inputs["nbr_idx"], np.int32)
    ew = np.asarray(inputs["edge_w"], np.float32)
    diag = np.asarray(inputs["diag"], np.float32)
    cw = np.asarray(inputs["cheb_w"], np.float32)
    cb = np.asarray(inputs["cheb_b"], np.float32)
    pw = np.asarray(inputs["pc_w"], np.float32)
    pb = np.asarray(inputs["pc_b"], np.float32)
    g1 = np.asarray(inputs["gamma1"], np.float32)
    b1 = np.asarray(inputs["beta1"], np.float32)
    g2 = np.asarray(inputs["gamma2"], np.float32)
    b2 = np.asarray(inputs["beta2"], np.float32)

    nc_, nsa, tpg = cfg.ncores, cfg.nsa, cfg.tpg
    x2 = np.ascontiguousarray(np.concatenate([x[0], x[1]], axis=1))  # [N, C]

    def shard_sup(arr, w):
        # [N, w] -> [ncores, 128, nsa*tpg*w]; col order: (supertile, tile, w)
        a = arr.reshape(nc_, nsa, tpg, P, w)
        a = a.transpose(0, 3, 1, 2, 4)
        return np.ascontiguousarray(a.reshape(nc_, P, nsa * tpg * w))

    idxs = shard_sup(nbr, NB)
    ew1s = shard_sup(ew, NB).astype(np.float16)
    ew2s = shard_sup(2.0 * ew, NB).astype(np.float16)
    dg1s = shard_sup(diag[:, None], 1)
    dg2s = shard_sup(2.0 * diag[:, None], 1)
    xs = x2.reshape(nc_, cfg.ns, C)

    cwT = np.zeros((K, 2, C, FOUT), np.float32)
    for k in range(K):
        for b in range(2):
            cwT[k, b, b * FIN : (b + 1) * FIN, :] = cw[k * FIN : (k + 1) * FIN, :]
    pwT = np.ascontiguousarray(pw.reshape(4, FOUT, FOUT))

    shared = {
        "cwT": cwT,
        "pwT": pwT,
        "cb": np.ascontiguousarray(cb.reshape(FOUT, 1)),
        "pb": np.ascontiguousarray(pb.reshape(FOUT, 1)),
    }
    if cfg.l1aff:
        shared["g1r"] = np.ascontiguousarray(np.broadcast_to(g1, (P, FOUT)))
        shared["b1r"] = np.ascontiguousarray(np.broadcast_to(b1, (P, FOUT)))
    if cfg.l2aff:
        shared["g2r"] = np.ascontiguousarray(np.broadcast_to(g2, (P, FOUT)))
        shared["b2r"] = np.ascontiguousarray(np.broadcast_to(b2, (P, FOUT)))

    in_maps = []
    for c in range(nc_):
        m = dict(shared)
        m["xs"] = np.ascontiguousarray(xs[c])
        m["idxs"] = idxs[c]
        m["ew1s"] = ew1s[c]
        m["ew2s"] = ew2s[c]
        m["dg1s"] = dg1s[c]
        m["dg2s"] = dg2s[c]
        in_maps.append(m)
    return in_maps


# ------------------------------------------------------------------ builder


def build_nc(cfg: Cfg):
    import concourse.bacc as bacc
    import concourse.bass as bass
    import concourse.mybir as mybir
    import concourse.tile as tile
    from concourse.masks import make_identity

    dt = mybir.dt
    f32 = dt.float32
    f16 = dt.float16
    i32 = dt.int32
    Alu = mybir.AluOpType
    Act = mybir.ActivationFunctionType
    Ax = mybir.AxisListType

    NS, nsa, nsm, tpg, mts = cfg.ns, cfg.nsa, cfg.nsm, cfg.tpg, cfg.mts
    TW = tpg * NB

    nc = bacc.Bacc(
        "TRN2",
        target_bir_lowering=False,
        debug=False,
        enable_asserts=False,
        num_devices=cfg.ncores,
    )

    xs = nc.dram_tensor("xs", [NS, C], f32, kind="ExternalInput")
    idxs_d = nc.dram_tensor("idxs", [P, nsa * TW], i32, kind="ExternalInput")
    ew1_d = nc.dram_tensor("ew1s", [P, nsa * TW], f16, kind="ExternalInput")
    ew2_d = nc.dram_tensor("ew2s", [P, nsa * TW], f16, kind="ExternalInput")
    dg1_d = nc.dram_tensor("dg1s", [P, nsa * tpg], f32, kind="ExternalInput")
    dg2_d = nc.dram_tensor("dg2s", [P, nsa * tpg], f32, kind="ExternalInput")
    cwT_d = nc.dram_tensor("cwT", [K, 2, C, FOUT], f32, kind="ExternalInput")
    pwT_d = nc.dram_tensor("pwT", [4, FOUT, FOUT], f32, kind="ExternalInput")
    cb_d = nc.dram_tensor("cb", [FOUT, 1], f32, kind="ExternalInput")
    pb_d = nc.dram_tensor("pb", [FOUT, 1], f32, kind="ExternalInput")
    aff_d = {}
    if cfg.l1aff:
        aff_d["g1r"] = nc.dram_tensor("g1r", [P, FOUT], f32, kind="ExternalInput")
        aff_d["b1r"] = nc.dram_tensor("b1r", [P, FOUT], f32, kind="ExternalInput")
    if cfg.l2aff:
        aff_d["g2r"] = nc.dram_tensor("g2r", [P, FOUT], f32, kind="ExternalInput")
        aff_d["b2r"] = nc.dram_tensor("b2r", [P, FOUT], f32, kind="ExternalInput")
    out_d = nc.dram_tensor("out", [B, NS // 4, FOUT], f32, kind="ExternalOutput")

    if cfg.noop:
        # Timing baseline: identical external I/O, minimal device work.
        with tile.TileContext(nc) as tc:
            with tc.tile_pool(name="sb", bufs=1) as sb:
                z = sb.tile([P, NS // 4], f32, name="z")
                nc.vector.memset(z[:], 0.0)
                for b in range(B):
                    nc.sync.dma_start(
                        out=out_d.ap()[b].rearrange("(t p) c -> p t c", p=P),
                        in_=z[:].rearrange("p (t c) -> p t c", c=FOUT),
                    )
        nc.compile()
        return nc

    rg = [list(range(cfg.ncores))]
    ph = cfg.phase
    do_apply = ph in ("all", "apply", "apply1")
    do_ag = ph in ("all", "ag")
    do_dense = ph in ("all", "dense")

    with tile.TileContext(nc) as tc:
        with (
            tc.tile_pool(name="const", bufs=1) as pc_,
            tc.tile_pool(name="dram", bufs=1, space="DRAM") as pdram,
        ):
            # persistent DRAM intermediates
            xsi = pdram.tile([NS, C], f16, name="xsi")
            t1s = pdram.tile([NS, C], f32, name="t1s")
            t2s = pdram.tile([NS, C], f32, name="t2s")
            t3s = pdram.tile([NS, C], f32, name="t3s")
            t1h = pdram.tile([NS, C], f16, name="t1h")
            t2h = pdram.tile([NS, C], f16, name="t2h")
            x2f = pdram.tile([cfg.n, C], f16, name="x2f", addr_space="Shared")
            t1f = pdram.tile([cfg.n, C], f16, name="t1f", addr_space="Shared")
            t2f = pdram.tile([cfg.n, C], f16, name="t2f", addr_space="Shared")

            # resident constants
            ident = pc_.tile([P, P], f32, name="ident")
            make_identity(nc, ident[:])
            idx_all = pc_.tile([P, nsa * TW], i32, name="idx_all")
            nc.sync.dma_start(out=idx_all[:], in_=idxs_d[:, :])
            ew1_all = pc_.tile([P, nsa * TW], f16, name="ew1_all")
            nc.sync.dma_start(out=ew1_all[:], in_=ew1_d[:, :])
            ew2_all = pc_.tile([P, nsa * TW], f16, name="ew2_all")
            nc.sync.dma_start(out=ew2_all[:], in_=ew2_d[:, :])
            dg1_all = pc_.tile([P, nsa * tpg], f32, name="dg1_all")
            nc.sync.dma_start(out=dg1_all[:], in_=dg1_d[:, :])
            dg2_all = pc_.tile([P, nsa * tpg], f32, name="dg2_all")
            nc.sync.dma_start(out=dg2_all[:], in_=dg2_d[:, :])
            cw_sb = []
            for k in range(K):
                row = []
                for b in range(2):
                    t = pc_.tile([C, FOUT], f32, name=f"cw_{k}_{b}")
                    nc.sync.dma_start(out=t[:], in_=cwT_d[k, b])
                    row.append(t)
                cw_sb.append(row)
            pw_sb = []
            for r in range(4):
                t = pc_.tile([FOUT, FOUT], f32, name=f"pw_{r}")
                nc.sync.dma_start(out=t[:], in_=pwT_d[r])
                pw_sb.append(t)
            cb_sb = pc_.tile([FOUT, 1], f32, name="cb_sb")
            nc.sync.dma_start(out=cb_sb[:], in_=cb_d[:, :])
            pb_sb = pc_.tile([FOUT, 1], f32, name="pb_sb")
            nc.sync.dma_start(out=pb_sb[:], in_=pb_d[:, :])
            aff_sb = {}
            for nm, d in aff_d.items():
                t = pc_.tile([P, FOUT], f32, name=f"{nm}_sb")
                nc.sync.dma_start(out=t[:], in_=d[:, :])
                aff_sb[nm] = t

            # stage x shard into an internal DRAM tile (collective ins must
            # be internal tiles, not kernel I/O), casting f32 -> f16 for the
            # gather table on the way through SBUF
            with tc.tile_pool(name="p_cast", bufs=3) as p_cast:
                for s in range(nsa):
                    rows = slice(s * tpg * P, (s + 1) * tpg * P)
                    xt = p_cast.tile([P, tpg * C], f32, name="xt", tag="xt")
                    nc.sync.dma_start(
                        out=xt[:].rearrange("p (t c) -> p t c", t=tpg),
                        in_=xs[rows, :].rearrange("(t p) c -> p t c", p=P),
                    )
                    xth = p_cast.tile([P, tpg * C], f16, name="xth", tag="xth")
                    nc.vector.tensor_copy(out=xth[:], in_=xt[:])
                    nc.sync.dma_start(
                        out=xsi[:, :][rows, :].rearrange("(t p) c -> p t c", p=P),
                        in_=xth[:].rearrange("p (t c) -> p t c", t=tpg),
                    )

            # ---------------- apply phase ----------------
            def emit_apply(
                src_full, own_cur, prev, ew_all, dg_all, dst, pools, dsth=None
            ):
                p_g, p_io = pools
                for s in range(nsa):
                    rows = slice(s * tpg * P, (s + 1) * tpg * P)
                    own = p_io.tile([P, tpg * C], f32, name="own", tag="own")
                    nc.sync.dma_start(
                        out=own[:].rearrange("p (t c) -> p t c", t=tpg),
                        in_=own_cur[rows, :].rearrange("(t p) c -> p t c", p=P),
                    )
                    if prev is not None:
                        prv = p_io.tile([P, tpg * C], f32, name="prv", tag="prv")
                        nc.sync.dma_start(
                            out=prv[:].rearrange("p (t c) -> p t c", t=tpg),
                            in_=prev[rows, :].rearrange("(t p) c -> p t c", p=P),
                        )
                    G = p_g.tile([P, TW * C], f16, name="G", tag="G")
                    for j in range(TW):
                        # one indirect DMA per 128 gathered rows (HW consumes
                        # exactly one offset per partition; batching the
                        # offset AP to [128, TW] passes MultiCoreSim but
                        # returns wrong data on real HW — verified 2026-08-09)
                        nc.gpsimd.indirect_dma_start(
                            out=G[:, j * C : (j + 1) * C],
                            out_offset=None,
                            in_=src_full,
                            in_offset=bass.IndirectOffsetOnAxis(
                                ap=idx_all[:, s * TW + j : s * TW + j + 1], axis=0
                            ),
                        )
                    tn = p_io.tile([P, tpg * C], f32, name="tn", tag="tn")
                    # G[:, (t k c)] *= ew[t, k], weight broadcast over c
                    Gv = G[:].rearrange("p (j c) -> p j c", j=TW)
                    nc.vector.tensor_tensor(
                        out=Gv,
                        in0=Gv,
                        in1=ew_all[:, s * TW : (s + 1) * TW].to_broadcast(
                            [P, TW, C]
                        ),
                        op=Alu.mult,
                    )
                    # tn[p, t, c] = sum_k G[p, t, k, c]  (innermost-axis reduce)
                    nc.vector.tensor_reduce(
                        tn[:],
                        G[:].rearrange("p (t k c) -> p t c k", t=tpg, k=NB),
                        axis=Ax.X,
                        op=Alu.add,
                    )
                    # own-term: od = own * diag (broadcast over c), then
                    # tn += od [- prev]
                    od = p_io.tile([P, tpg * C], f32, name="od", tag="od")
                    nc.vector.tensor_tensor(
                        out=od[:].rearrange("p (t c) -> p t c", t=tpg),
                        in0=own[:].rearrange("p (t c) -> p t c", t=tpg),
                        in1=dg_all[:, s * tpg : (s + 1) * tpg].to_broadcast(
                            [P, tpg, C]
                        ),
                        op=Alu.mult,
                    )
                    nc.vector.tensor_tensor(
                        out=tn[:], in0=tn[:], in1=od[:], op=Alu.add
                    )
                    if prev is not None:
                        nc.vector.tensor_tensor(
                            out=tn[:], in0=tn[:], in1=prv[:], op=Alu.subtract
                        )
                    nc.sync.dma_start(
                        out=dst[rows, :].rearrange("(t p) c -> p t c", p=P),
                        in_=tn[:].rearrange("p (t c) -> p t c", t=tpg),
                    )
                    if dsth is not None:
                        tnh = p_io.tile([P, tpg * C], f16, name="tnh", tag="tnh")
                        nc.vector.tensor_copy(out=tnh[:], in_=tn[:])
                        nc.sync.dma_start(
                            out=dsth[rows, :].rearrange("(t p) c -> p t c", p=P),
                            in_=tnh[:].rearrange("p (t c) -> p t c", t=tpg),
                        )

            def ag(src, dst):
                nc.gpsimd.collective_compute(
                    "AllGather",
                    Alu.bypass,
                    replica_groups=rg,
                    ins=[src.opt()],
                    outs=[dst.opt()],
                )

            with (
                tc.tile_pool(name="p_g", bufs=3) as p_g,
                tc.tile_pool(name="p_io", bufs=4) as p_io,
            ):
                pools = (p_g, p_io)
                if do_ag:
                    ag(xsi, x2f)
                if do_apply:
                    emit_apply(
                        x2f[:, :], xs[:, :], None, ew1_all, dg1_all, t1s[:, :],
                        pools, dsth=t1h[:, :],
                    )
                if do_ag:
                    ag(t1h, t1f)
                if do_apply and ph != "apply1":
                    emit_apply(
                        t1f[:, :], t1s[:, :], xs[:, :], ew2_all, dg2_all,
                        t2s[:, :], pools, dsth=t2h[:, :],
                    )
                if do_ag:
                    ag(t2h, t2f)
                if do_apply and ph != "apply1":
                    emit_apply(
                        t2f[:, :], t2s[:, :], t1s[:, :], ew2_all, dg2_all,
                        t3s[:, :], pools,
                    )

            # ---------------- dense phase ----------------
            def emit_ln(psr, out_sbuf, gb_tuple, p_small, p_scr):
                # psr: [128, FOUT] row-major (PSUM); out_sbuf: [128, FOUT] SBUF
                ssum = p_small.tile([P, 1], f32, name="ssum", tag="ssum")
                nc.vector.tensor_reduce(ssum[:], psr, axis=Ax.X, op=Alu.add)
                negmu = p_small.tile([P, 1], f32, name="negmu", tag="negmu")
                nc.vector.tensor_scalar_mul(negmu[:], ssum[:], -1.0 / FOUT)
                sq = p_scr.tile([P, FOUT], f32, name="sq", tag="sq")
                vp0 = p_small.tile([P, 1], f32, name="vp0", tag="vp0")
                nc.scalar.activation(
                    sq[:],
                    psr,
                    Act.Square,
                    bias=negmu[:, :1],
                    scale=1.0,
                    accum_out=vp0[:],
                )
                vpe = p_small.tile([P, 1], f32, name="vpe", tag="vpe")
                nc.vector.tensor_scalar(
                    vpe[:], vp0[:], 1.0 / FOUT, EPS, op0=Alu.mult, op1=Alu.add
                )
                sig = p_small.tile([P, 1], f32, name="sig", tag="sig")
                nc.scalar.sqrt(sig[:], vpe[:])
                rsig = p_small.tile([P, 1], f32, name="rsig", tag="rsig")
                nc.vector.reciprocal(rsig[:], sig[:])
                nmr = p_small.tile([P, 1], f32, name="nmr", tag="nmr")
                nc.vector.tensor_tensor(
                    out=nmr[:], in0=negmu[:], in1=rsig[:], op=Alu.mult
                )
                if gb_tuple is None:
                    nc.scalar.activation(
                        out_sbuf, psr, Act.Identity, bias=nmr[:, :1],
                        scale=rsig[:, :1],
                    )
                else:
                    gtmp = p_scr.tile([P, FOUT], f32, name="gtmp", tag="gtmp")
                    nc.scalar.activation(
                        gtmp[:], psr, Act.Identity, bias=nmr[:, :1],
                        scale=rsig[:, :1],
                    )
                    gb, bb = gb_tuple
                    nc.vector.tensor_tensor(
                        out=gtmp[:], in0=gtmp[:], in1=gb[:], op=Alu.mult
                    )
                    nc.vector.tensor_tensor(
                        out=out_sbuf, in0=gtmp[:], in1=bb[:], op=Alu.add
                    )

            ln1_gb = (aff_sb["g1r"], aff_sb["b1r"]) if cfg.l1aff else None
            ln2_gb = (aff_sb["g2r"], aff_sb["b2r"]) if cfg.l2aff else None

            if not do_dense:
                with tc.tile_pool(name="zo", bufs=1) as zo:
                    z = zo.tile([P, NS // 4], f32, name="z")
                    nc.vector.memset(z[:], 0.0)
                    for b in range(B):
                        nc.sync.dma_start(
                            out=out_d.ap()[b].rearrange("(t p) c -> p t c", p=P),
                            in_=z[:].rearrange("p (t c) -> p t c", c=FOUT),
                        )

            if do_dense:
                with (
                    tc.tile_pool(name="p_ld", bufs=4) as p_ld,
                    tc.tile_pool(name="p_tt", bufs=2) as p_tt,
                    tc.tile_pool(name="p_h1", bufs=2) as p_h1,
                    tc.tile_pool(name="p_cs", bufs=4) as p_cs,
                    tc.tile_pool(name="p_small", bufs=8) as p_small,
                    tc.tile_pool(name="p_psT", bufs=4, space="PSUM") as p_psT,
                    tc.tile_pool(name="p_psM", bufs=4, space="PSUM") as p_psM,
                ):
                    srcs = [xs[:, :], t1s[:, :], t2s[:, :], t3s[:, :]]
                    for m in range(nsm):
                        # stage A: load + transpose T_k tiles -> TkT [C, mts*128]
                        TkT = []
                        for k in range(K):
                            tkt = p_tt.tile(
                                [P, mts * P], f32, name=f"TkT{k}", tag=f"TkT{k}"
                            )
                            for t0 in range(0, mts, 4):
                                pst = p_psT.tile(
                                    [P, 4 * P], f32, name="pst", tag="pst",
                                    space="PSUM",
                                )
                                r0 = m * mts * P + t0 * P
                                ld4 = p_ld.tile([P, 4 * P], f32, name="ld", tag="ld")
                                nc.sync.dma_start(
                                    out=ld4[:].rearrange("p (t c) -> p t c", t=4),
                                    in_=srcs[k][r0 : r0 + 4 * P, :].rearrange(
                                        "(t p) c -> p t c", p=P
                                    ),
                                )
                                for i in range(4):
                                    nc.tensor.transpose(
                                        pst[:, i * P : (i + 1) * P],
                                        ld4[:, i * P : (i + 1) * P],
                                        ident[:],
                                    )
                                nc.vector.tensor_copy(
                                    out=tkt[:, t0 * P : (t0 + 4) * P], in_=pst[:]
                                )
                            TkT.append(tkt)
                        # stage B: cheb matmul -> hps[b][half] [FOUT, 512] psum
                        hps = [[None, None], [None, None]]
                        for b in range(2):
                            for h in range(mts * P // 512):
                                ps = p_psM.tile(
                                    [P, 512], f32, name="hps", tag="hps",
                                    space="PSUM",
                                )
                                hps[b][h] = ps
                                for k in range(K):
                                    nc.tensor.matmul(
                                        ps[:],
                                        cw_sb[k][b][:],
                                        TkT[k][:, h * 512 : (h + 1) * 512],
                                        start=(k == 0),
                                        stop=(k == K - 1),
                                    )
                        # stage C: relu+LN1 per [128] target tile; build h1T
                        h1T = []
                        for b in range(2):
                            h1t = p_h1.tile(
                                [P, mts * P], f32, name=f"h1T{b}", tag=f"h1T{b}"
                            )
                            hTs_l = []
                            for h in range(mts * P // 512):
                                hTs = p_cs.tile([P, 512], f32, name="hTs", tag="hTs")
                                nc.scalar.activation(
                                    hTs[:],
                                    hps[b][h][:],
                                    Act.Relu,
                                    bias=cb_sb[:, :1],
                                    scale=1.0,
                                )
                                hTs_l.append(hTs)
                            for t0 in range(0, mts, 4):
                                psb4 = p_psT.tile(
                                    [P, 4 * P], f32, name="psb4", tag="pst",
                                    space="PSUM",
                                )
                                for i in range(4):
                                    t = t0 + i
                                    h, off = divmod(t * P, 512)
                                    psr = p_psT.tile(
                                        [P, P], f32, name="psr", tag="pst",
                                        space="PSUM",
                                    )
                                    nc.tensor.transpose(
                                        psr[:], hTs_l[h][:, off : off + P], ident[:]
                                    )
                                    h1row = p_cs.tile(
                                        [P, P], f32, name="h1row", tag="h1row"
                                    )
                                    emit_ln(psr[:], h1row[:], ln1_gb, p_small, p_cs)
                                    nc.tensor.transpose(
                                        psb4[:, i * P : (i + 1) * P], h1row[:],
                                        ident[:],
                                    )
                                nc.vector.tensor_copy(
                                    out=h1t[:, t0 * P : (t0 + 4) * P], in_=psb4[:]
                                )
                            h1T.append(h1t)
                        # stage D: pseudo-conv matmul -> h2ps[b] [FOUT, mts*32]
                        noc = mts * P // 4  # out targets per supertile
                        h2ps = []
                        for b in range(2):
                            ps = p_psM.tile(
                                [P, noc], f32, name="h2ps", tag="hps", space="PSUM"
                            )
                            rview = h1T[b][:].rearrange("p (i r) -> p r i", r=4)
                            for r in range(4):
                                nc.tensor.matmul(
                                    ps[:],
                                    pw_sb[r][:],
                                    rview[:, r, :],
                                    start=(r == 0),
                                    stop=(r == 3),
                                )
                            h2ps.append(ps)
                        # stage E: relu+LN2, write out
                        for b in range(2):
                            h2Ts = p_cs.tile([P, noc], f32, name="h2Ts", tag="hTs")
                            nc.scalar.activation(
                                h2Ts[:],
                                h2ps[b][:],
                                Act.Relu,
                                bias=pb_sb[:, :1],
                                scale=1.0,
                            )
                            for j in range(noc // P):
                                psr2 = p_psT.tile(
                                    [P, P], f32, name="psr2", tag="pst",
                                    space="PSUM",
                                )
                                nc.tensor.transpose(
                                    psr2[:], h2Ts[:, j * P : (j + 1) * P], ident[:]
                                )
                                orow = p_cs.tile([P, P], f32, name="orow", tag="orow")
                                emit_ln(psr2[:], orow[:], ln2_gb, p_small, p_cs)
                                o0 = m * (mts * P // 4) + j * P
                                nc.sync.dma_start(
                                    out=out_d.ap()[b, o0 : o0 + P, :], in_=orow[:]
                                )

    nc.compile()
    return nc


# ------------------------------------------------------------------ entry


@functools.lru_cache(maxsize=4)
def _compiled(cfg: Cfg):
    return build_nc(cfg)


def kernel(**inputs) -> np.ndarray:
    from concourse.bass_utils import run_bass_kernel_spmd

    n = inputs["x"].shape[1]
    cfg = dataclasses.replace(
        FULL,
        n=n,
        l1aff=not (
            np.all(np.asarray(inputs["gamma1"]) == 1.0)
            and np.all(np.asarray(inputs["beta1"]) == 0.0)
        ),
        l2aff=not (
            np.all(np.asarray(inputs["gamma2"]) == 1.0)
            and np.all(np.asarray(inputs["beta2"]) == 0.0)
        ),
    )
    nc = _compiled(cfg)
    in_maps = host_prep(inputs, cfg)
    res = run_bass_kernel_spmd(nc, in_maps, list(range(cfg.ncores)))
    out = np.concatenate([res.results[i]["out"] for i in range(cfg.ncores)], axis=1)
    return np.ascontiguousarray(out)


# revision 10
# speedup vs baseline: 247.3489x; 1.0999x over previous
"""Trainium2 Bass kernel for nn_MiddleBlock (Chebyshev graph conv + pseudo-conv).

Reference computation (B=2, N=196608, FIN=64, K=4, FOUT=128, NB=8):
  T0 = x; T1 = L x; T_k = 2 L T_{k-1} - T_{k-2}   with
  (L y)[i] = sum_k edge_w[i,k] * y[nbr[i,k]] + diag[i] * y[i]
  h = relu(concat(T0..T3) @ cheb_w + cheb_b); h = LN(h)
  h = relu(h.reshape(B, N/4, 4*FOUT) @ pc_w + pc_b); h = LN(h)

Distribution over 8 NeuronCores: both batches are fused into the column dim
(col = b*64 + f, 512B rows) and the pixel axis N is sharded 8 ways.  Each
core receives only its own shard of x (no host-side replication); the full
gather table for each term is built on-device with an AllGather.  Each
apply_L gathers neighbor rows from the replicated table via indirect DMA;
the dense phase (cheb matmul, LN, pseudo-conv, LN) is fully local per shard.
"""

import dataclasses
import functools

import numpy as np

B, FIN, K, FOUT, NB = 2, 64, 4, 128, 8
C = B * FIN  # fused column dim = 128
EPS = 1e-6
P = 128


@dataclasses.dataclass(frozen=True)
class Cfg:
    n: int = 196608
    ncores: int = 8
    tpg: int = 8  # target tiles per gather supertile (apply phase)
    mts: int = 8  # target tiles per matmul supertile (must be mult of 4)
    l1aff: bool = False  # apply gamma1/beta1
    l2aff: bool = False
    noop: bool = False  # timing baseline: same I/O, no compute
    phase: str = "all"  # all | apply | apply1 | ag | dense  (perf bisection)

    @property
    def ns(self):
        return self.n // self.ncores

    @property
    def nsa(self):  # apply supertiles per core
        return self.ns // (self.tpg * P)

    @property
    def nsm(self):  # matmul supertiles per core
        return self.ns // (self.mts * P)


FULL = Cfg()


# ------------------------------------------------------------------ host prep


def host_prep(inputs: dict, cfg: Cfg) -> list[dict]:
    """Build per-core input maps for the SPMD kernel."""
    x = np.asarray(inputs["x"], np.float32)
    nbr = np.asarray(inputs["nbr_idx"], np.int32)
    ew = np.asarray(inputs["edge_w"], np.float32)
    diag = np.asarray(inputs["diag"], np.float32)
    cw = np.asarray(inputs["cheb_w"], np.float32)
    cb = np.asarray(inputs["cheb_b"], np.float32)
    pw = np.asarray(inputs["pc_w"], np.float32)
    pb = np.asarray(inputs["pc_b"], np.float32)
    g1 = np.asarray(inputs["gamma1"], np.float32)
    b1 = np.asarray(inputs["beta1"], np.float32)
    g2 = np.asarray(inputs["gamma2"], np.float32)
    b2 = np.asarray(inputs["beta2"], np.float32)

    nc_, nsa, tpg = cfg.ncores, cfg.nsa, cfg.tpg
    x2 = np.ascontiguousarray(np.concatenate([x[0], x[1]], axis=1))  # [N, C]

    def shard_sup(arr, w):
        # [N, w] -> [ncores, 128, nsa*tpg*w]; col order: (supertile, tile, w)
        a = arr.reshape(nc_, nsa, tpg, P, w)
        a = a.transpose(0, 3, 1, 2, 4)
        return np.ascontiguousarray(a.reshape(nc_, P, nsa * tpg * w))

    idxs = shard_sup(nbr, NB)
    ew1s = shard_sup(ew, NB).astype(np.float16)
    ew2s = shard_sup(2.0 * ew, NB).astype(np.float16)
    dg1s = shard_sup(diag[:, None], 1)
    dg2s = shard_sup(2.0 * diag[:, None], 1)
    xs = x2.reshape(nc_, cfg.ns, C)

    cwT = np.zeros((K, 2, C, FOUT), np.float32)
    for k in range(K):
        for b in range(2):
            cwT[k, b, b * FIN : (b + 1) * FIN, :] = cw[k * FIN : (k + 1) * FIN, :]
    pwT = np.ascontiguousarray(pw.reshape(4, FOUT, FOUT))

    shared = {
        "cwT": cwT,
        "pwT": pwT,
        "cb": np.ascontiguousarray(cb.reshape(FOUT, 1)),
        "pb": np.ascontiguousarray(pb.reshape(FOUT, 1)),
    }
    if cfg.l1aff:
        shared["g1r"] = np.ascontiguousarray(np.broadcast_to(g1, (P, FOUT)))
        shared["b1r"] = np.ascontiguousarray(np.broadcast_to(b1, (P, FOUT)))
    if cfg.l2aff:
        shared["g2r"] = np.ascontiguousarray(np.broadcast_to(g2, (P, FOUT)))
        shared["b2r"] = np.ascontiguousarray(np.broadcast_to(b2, (P, FOUT)))

    in_maps = []
    for c in range(nc_):
        m = dict(shared)
        m["xs"] = np.ascontiguousarray(xs[c])
        m["idxs"] = idxs[c]
        m["ew1s"] = ew1s[c]
        m["ew2s"] = ew2s[c]
        m["dg1s"] = dg1s[c]
        m["dg2s"] = dg2s[c]
        in_maps.append(m)
    return in_maps


# ------------------------------------------------------------------ builder


def build_nc(cfg: Cfg):
    import concourse.bacc as bacc
    import concourse.bass as bass
    import concourse.mybir as mybir
    import concourse.tile as tile
    from concourse.masks import make_identity

    dt = mybir.dt
    f32 = dt.float32
    f16 = dt.float16
    i32 = dt.int32
    Alu = mybir.AluOpType
    Act = mybir.ActivationFunctionType
    Ax = mybir.AxisListType

    NS, nsa, nsm, tpg, mts = cfg.ns, cfg.nsa, cfg.nsm, cfg.tpg, cfg.mts
    TW = tpg * NB

    nc = bacc.Bacc(
        "TRN2",
        target_bir_lowering=False,
        debug=False,
        enable_asserts=False,
        num_devices=cfg.ncores,
    )

    xs = nc.dram_tensor("xs", [NS, C], f32, kind="ExternalInput")
    idxs_d = nc.dram_tensor("idxs", [P, nsa * TW], i32, kind="ExternalInput")
    ew1_d = nc.dram_tensor("ew1s", [P, nsa * TW], f16, kind="ExternalInput")
    ew2_d = nc.dram_tensor("ew2s", [P, nsa * TW], f16, kind="ExternalInput")
    dg1_d = nc.dram_tensor("dg1s", [P, nsa * tpg], f32, kind="ExternalInput")
    dg2_d = nc.dram_tensor("dg2s", [P, nsa * tpg], f32, kind="ExternalInput")
    cwT_d = nc.dram_tensor("cwT", [K, 2, C, FOUT], f32, kind="ExternalInput")
    pwT_d = nc.dram_tensor("pwT", [4, FOUT, FOUT], f32, kind="ExternalInput")
    cb_d = nc.dram_tensor("cb", [FOUT, 1], f32, kind="ExternalInput")
    pb_d = nc.dram_tensor("pb", [FOUT, 1], f32, kind="ExternalInput")
    aff_d = {}
    if cfg.l1aff:
        aff_d["g1r"] = nc.dram_tensor("g1r", [P, FOUT], f32, kind="ExternalInput")
        aff_d["b1r"] = nc.dram_tensor("b1r", [P, FOUT], f32, kind="ExternalInput")
    if cfg.l2aff:
        aff_d["g2r"] = nc.dram_tensor("g2r", [P, FOUT], f32, kind="ExternalInput")
        aff_d["b2r"] = nc.dram_tensor("b2r", [P, FOUT], f32, kind="ExternalInput")
    out_d = nc.dram_tensor("out", [B, NS // 4, FOUT], f32, kind="ExternalOutput")

    if cfg.noop:
        # Timing baseline: identical external I/O, minimal device work.
        with tile.TileContext(nc) as tc:
            with tc.tile_pool(name="sb", bufs=1) as sb:
                z = sb.tile([P, NS // 4], f32, name="z")
                nc.vector.memset(z[:], 0.0)
                for b in range(B):
                    nc.sync.dma_start(
                        out=out_d.ap()[b].rearrange("(t p) c -> p t c", p=P),
                        in_=z[:].rearrange("p (t c) -> p t c", c=FOUT),
                    )
        nc.compile()
        return nc

    rg = [list(range(cfg.ncores))]
    ph = cfg.phase
    do_apply = ph in ("all", "apply", "apply1")
    do_ag = ph in ("all", "ag")
    do_dense = ph in ("all", "dense")

    with tile.TileContext(nc) as tc:
        with (
            tc.tile_pool(name="const", bufs=1) as pc_,
            tc.tile_pool(name="dram", bufs=1, space="DRAM") as pdram,
        ):
            # persistent DRAM intermediates
            xsi = pdram.tile([NS, C], f16, name="xsi")
            t1s = pdram.tile([NS, C], f32, name="t1s")
            t2s = pdram.tile([NS, C], f32, name="t2s")
            t3s = pdram.tile([NS, C], f32, name="t3s")
            t1h = pdram.tile([NS, C], f16, name="t1h")
            t2h = pdram.tile([NS, C], f16, name="t2h")
            x2f = pdram.tile([cfg.n, C], f16, name="x2f", addr_space="Shared")
            t1f = pdram.tile([cfg.n, C], f16, name="t1f", addr_space="Shared")
            t2f = pdram.tile([cfg.n, C], f16, name="t2f", addr_space="Shared")

            # resident constants
            ident = pc_.tile([P, P], f32, name="ident")
            make_identity(nc, ident[:])
            idx_all = pc_.tile([P, nsa * TW], i32, name="idx_all")
            nc.sync.dma_start(out=idx_all[:], in_=idxs_d[:, :])
            ew1_all = pc_.tile([P, nsa * TW], f16, name="ew1_all")
            nc.sync.dma_start(out=ew1_all[:], in_=ew1_d[:, :])
            ew2_all = pc_.tile([P, nsa * TW], f16, name="ew2_all")
            nc.sync.dma_start(out=ew2_all[:], in_=ew2_d[:, :])
            dg1_all = pc_.tile([P, nsa * tpg], f32, name="dg1_all")
            nc.sync.dma_start(out=dg1_all[:], in_=dg1_d[:, :])
            dg2_all = pc_.tile([P, nsa * tpg], f32, name="dg2_all")
            nc.sync.dma_start(out=dg2_all[:], in_=dg2_d[:, :])
            cw_sb = []
            for k in range(K):
                row = []
                for b in range(2):
                    t = pc_.tile([C, FOUT], f32, name=f"cw_{k}_{b}")
                    nc.sync.dma_start(out=t[:], in_=cwT_d[k, b])
                    row.append(t)
                cw_sb.append(row)
            pw_sb = []
            for r in range(4):
                t = pc_.tile([FOUT, FOUT], f32, name=f"pw_{r}")
                nc.sync.dma_start(out=t[:], in_=pwT_d[r])
                pw_sb.append(t)
            cb_sb = pc_.tile([FOUT, 1], f32, name="cb_sb")
            nc.sync.dma_start(out=cb_sb[:], in_=cb_d[:, :])
            pb_sb = pc_.tile([FOUT, 1], f32, name="pb_sb")
            nc.sync.dma_start(out=pb_sb[:], in_=pb_d[:, :])
            aff_sb = {}
            for nm, d in aff_d.items():
                t = pc_.tile([P, FOUT], f32, name=f"{nm}_sb")
                nc.sync.dma_start(out=t[:], in_=d[:, :])
                aff_sb[nm] = t

            # stage x shard into an internal DRAM tile (collective ins must
            # be internal tiles, not kernel I/O), casting f32 -> f16 for the
            # gather table on the way through SBUF
            with tc.tile_pool(name="p_cast", bufs=3) as p_cast:
                for s in range(nsa):
                    rows = slice(s * tpg * P, (s + 1) * tpg * P)
                    xt = p_cast.tile([P, tpg * C], f32, name="xt", tag="xt")
                    nc.sync.dma_start(
                        out=xt[:].rearrange("p (t c) -> p t c", t=tpg),
                        in_=xs[rows, :].rearrange("(t p) c -> p t c", p=P),
                    )
                    xth = p_cast.tile([P, tpg * C], f16, name="xth", tag="xth")
                    nc.vector.tensor_copy(out=xth[:], in_=xt[:])
                    nc.sync.dma_start(
                        out=xsi[:, :][rows, :].rearrange("(t p) c -> p t c", p=P),
                        in_=xth[:].rearrange("p (t c) -> p t c", t=tpg),
                    )

            # ---------------- apply phase ----------------
            def emit_apply_sup(
                s, src_full, own_cur, prev, ew_all, dg_all, pools
            ):
                p_g, p_io = pools
                if True:
                    rows = slice(s * tpg * P, (s + 1) * tpg * P)
                    own = p_io.tile([P, tpg * C], f32, name="own", tag="own")
                    nc.sync.dma_start(
                        out=own[:].rearrange("p (t c) -> p t c", t=tpg),
                        in_=own_cur[rows, :].rearrange("(t p) c -> p t c", p=P),
                    )
                    if prev is not None:
                        prv = p_io.tile([P, tpg * C], f32, name="prv", tag="prv")
                        nc.sync.dma_start(
                            out=prv[:].rearrange("p (t c) -> p t c", t=tpg),
                            in_=prev[rows, :].rearrange("(t p) c -> p t c", p=P),
                        )
                    G = p_g.tile([P, TW * C], f16, name="G", tag="G")
                    for j in range(TW):
                        # one indirect DMA per 128 gathered rows (HW consumes
                        # exactly one offset per partition; batching the
                        # offset AP to [128, TW] passes MultiCoreSim but
                        # returns wrong data on real HW — verified 2026-08-09)
                        nc.gpsimd.indirect_dma_start(
                            out=G[:, j * C : (j + 1) * C],
                            out_offset=None,
                            in_=src_full,
                            in_offset=bass.IndirectOffsetOnAxis(
                                ap=idx_all[:, s * TW + j : s * TW + j + 1], axis=0
                            ),
                        )
                    tn = p_io.tile([P, tpg * C], f32, name="tn", tag="tn")
                    # G[:, (t k c)] *= ew[t, k], weight broadcast over c
                    Gv = G[:].rearrange("p (j c) -> p j c", j=TW)
                    nc.vector.tensor_tensor(
                        out=Gv,
                        in0=Gv,
                        in1=ew_all[:, s * TW : (s + 1) * TW].to_broadcast(
                            [P, TW, C]
                        ),
                        op=Alu.mult,
                    )
                    # tn[p, t, c] = sum_k G[p, t, k, c]  (innermost-axis reduce)
                    nc.vector.tensor_reduce(
                        tn[:],
                        G[:].rearrange("p (t k c) -> p t c k", t=tpg, k=NB),
                        axis=Ax.X,
                        op=Alu.add,
                    )
                    # own-term: od = own * diag (broadcast over c), then
                    # tn += od [- prev]
                    od = p_io.tile([P, tpg * C], f32, name="od", tag="od")
                    nc.vector.tensor_tensor(
                        out=od[:].rearrange("p (t c) -> p t c", t=tpg),
                        in0=own[:].rearrange("p (t c) -> p t c", t=tpg),
                        in1=dg_all[:, s * tpg : (s + 1) * tpg].to_broadcast(
                            [P, tpg, C]
                        ),
                        op=Alu.mult,
                    )
                    nc.vector.tensor_tensor(
                        out=tn[:], in0=tn[:], in1=od[:], op=Alu.add
                    )
                    if prev is not None:
                        nc.vector.tensor_tensor(
                            out=tn[:], in0=tn[:], in1=prv[:], op=Alu.subtract
                        )
                    return tn

            def emit_apply(
                src_full, own_cur, prev, ew_all, dg_all, dst, pools, dsth=None
            ):
                p_g, p_io = pools
                for s in range(nsa):
                    rows = slice(s * tpg * P, (s + 1) * tpg * P)
                    tn = emit_apply_sup(
                        s, src_full, own_cur, prev, ew_all, dg_all, pools
                    )
                    nc.sync.dma_start(
                        out=dst[rows, :].rearrange("(t p) c -> p t c", p=P),
                        in_=tn[:].rearrange("p (t c) -> p t c", t=tpg),
                    )
                    if dsth is not None:
                        tnh = p_io.tile([P, tpg * C], f16, name="tnh", tag="tnh")
                        nc.vector.tensor_copy(out=tnh[:], in_=tn[:])
                        nc.sync.dma_start(
                            out=dsth[rows, :].rearrange("(t p) c -> p t c", p=P),
                            in_=tnh[:].rearrange("p (t c) -> p t c", t=tpg),
                        )

            def ag(src, dst):
                nc.gpsimd.collective_compute(
                    "AllGather",
                    Alu.bypass,
                    replica_groups=rg,
                    ins=[src.opt()],
                    outs=[dst.opt()],
                )

            with (
                tc.tile_pool(name="p_g", bufs=3) as p_g,
                tc.tile_pool(name="p_io", bufs=4) as p_io,
            ):
                pools = (p_g, p_io)
                if do_ag:
                    ag(xsi, x2f)
                if do_apply:
                    emit_apply(
                        x2f[:, :], xs[:, :], None, ew1_all, dg1_all, t1s[:, :],
                        pools, dsth=t1h[:, :],
                    )
                if do_ag:
                    ag(t1h, t1f)
                if do_apply and ph != "apply1":
                    emit_apply(
                        t1f[:, :], t1s[:, :], xs[:, :], ew2_all, dg2_all,
                        t2s[:, :], pools, dsth=t2h[:, :],
                    )
                if do_ag:
                    ag(t2h, t2f)
                if ph == "apply":
                    emit_apply(
                        t2f[:, :], t2s[:, :], t1s[:, :], ew2_all, dg2_all,
                        t3s[:, :], pools,
                    )

            # ---------------- dense phase ----------------
            def emit_ln(psr, out_sbuf, gb_tuple, p_small, p_scr):
                # psr: [128, FOUT] row-major (PSUM); out_sbuf: [128, FOUT] SBUF
                ssum = p_small.tile([P, 1], f32, name="ssum", tag="ssum")
                nc.vector.tensor_reduce(ssum[:], psr, axis=Ax.X, op=Alu.add)
                negmu = p_small.tile([P, 1], f32, name="negmu", tag="negmu")
                nc.vector.tensor_scalar_mul(negmu[:], ssum[:], -1.0 / FOUT)
                sq = p_scr.tile([P, FOUT], f32, name="sq", tag="sq")
                vp0 = p_small.tile([P, 1], f32, name="vp0", tag="vp0")
                nc.scalar.activation(
                    sq[:],
                    psr,
                    Act.Square,
                    bias=negmu[:, :1],
                    scale=1.0,
                    accum_out=vp0[:],
                )
                vpe = p_small.tile([P, 1], f32, name="vpe", tag="vpe")
                nc.vector.tensor_scalar(
                    vpe[:], vp0[:], 1.0 / FOUT, EPS, op0=Alu.mult, op1=Alu.add
                )
                sig = p_small.tile([P, 1], f32, name="sig", tag="sig")
                nc.scalar.sqrt(sig[:], vpe[:])
                rsig = p_small.tile([P, 1], f32, name="rsig", tag="rsig")
                nc.vector.reciprocal(rsig[:], sig[:])
                nmr = p_small.tile([P, 1], f32, name="nmr", tag="nmr")
                nc.vector.tensor_tensor(
                    out=nmr[:], in0=negmu[:], in1=rsig[:], op=Alu.mult
                )
                if gb_tuple is None:
                    nc.scalar.activation(
                        out_sbuf, psr, Act.Identity, bias=nmr[:, :1],
                        scale=rsig[:, :1],
                    )
                else:
                    gtmp = p_scr.tile([P, FOUT], f32, name="gtmp", tag="gtmp")
                    nc.scalar.activation(
                        gtmp[:], psr, Act.Identity, bias=nmr[:, :1],
                        scale=rsig[:, :1],
                    )
                    gb, bb = gb_tuple
                    nc.vector.tensor_tensor(
                        out=gtmp[:], in0=gtmp[:], in1=gb[:], op=Alu.mult
                    )
                    nc.vector.tensor_tensor(
                        out=out_sbuf, in0=gtmp[:], in1=bb[:], op=Alu.add
                    )

            ln1_gb = (aff_sb["g1r"], aff_sb["b1r"]) if cfg.l1aff else None
            ln2_gb = (aff_sb["g2r"], aff_sb["b2r"]) if cfg.l2aff else None

            if not do_dense:
                with tc.tile_pool(name="zo", bufs=1) as zo:
                    z = zo.tile([P, NS // 4], f32, name="z")
                    nc.vector.memset(z[:], 0.0)
                    for b in range(B):
                        nc.sync.dma_start(
                            out=out_d.ap()[b].rearrange("(t p) c -> p t c", p=P),
                            in_=z[:].rearrange("p (t c) -> p t c", c=FOUT),
                        )

            if do_dense:
                with (
                    tc.tile_pool(name="p_g2", bufs=2) as p_g2,
                    tc.tile_pool(name="p_io2", bufs=2) as p_io2,
                    tc.tile_pool(name="p_ld", bufs=4) as p_ld,
                    tc.tile_pool(name="p_tt", bufs=2) as p_tt,
                    tc.tile_pool(name="p_h1", bufs=2) as p_h1,
                    tc.tile_pool(name="p_cs", bufs=4) as p_cs,
                    tc.tile_pool(name="p_small", bufs=8) as p_small,
                    tc.tile_pool(name="p_psT", bufs=4, space="PSUM") as p_psT,
                    tc.tile_pool(name="p_psM", bufs=4, space="PSUM") as p_psM,
                ):
                    srcs = [xs[:, :], t1s[:, :], t2s[:, :]]
                    for m in range(nsm):
                        # fused apply3 for this supertile -> tn3 [P, tpg*C]
                        tn3 = emit_apply_sup(
                            m, t2f[:, :], t2s[:, :], t1s[:, :], ew2_all,
                            dg2_all, (p_g2, p_io2),
                        )
                        # stage A: load + transpose T_k tiles -> TkT [C, mts*128]
                        TkT = []
                        for k in range(K - 1):
                            tkt = p_tt.tile(
                                [P, mts * P], f32, name=f"TkT{k}", tag=f"TkT{k}"
                            )
                            for t0 in range(0, mts, 4):
                                pst = p_psT.tile(
                                    [P, 4 * P], f32, name="pst", tag="pst",
                                    space="PSUM",
                                )
                                r0 = m * mts * P + t0 * P
                                ld4 = p_ld.tile([P, 4 * P], f32, name="ld", tag="ld")
                                nc.sync.dma_start(
                                    out=ld4[:].rearrange("p (t c) -> p t c", t=4),
                                    in_=srcs[k][r0 : r0 + 4 * P, :].rearrange(
                                        "(t p) c -> p t c", p=P
                                    ),
                                )
                                for i in range(4):
                                    nc.tensor.transpose(
                                        pst[:, i * P : (i + 1) * P],
                                        ld4[:, i * P : (i + 1) * P],
                                        ident[:],
                                    )
                                nc.vector.tensor_copy(
                                    out=tkt[:, t0 * P : (t0 + 4) * P], in_=pst[:]
                                )
                            TkT.append(tkt)
                        tkt3 = p_tt.tile(
                            [P, mts * P], f32, name="TkT3", tag="TkT3"
                        )
                        for t0 in range(0, mts, 4):
                            pst = p_psT.tile(
                                [P, 4 * P], f32, name="pst", tag="pst",
                                space="PSUM",
                            )
                            for i in range(4):
                                nc.tensor.transpose(
                                    pst[:, i * P : (i + 1) * P],
                                    tn3[:, (t0 + i) * P : (t0 + i + 1) * P],
                                    ident[:],
                                )
                            nc.vector.tensor_copy(
                                out=tkt3[:, t0 * P : (t0 + 4) * P], in_=pst[:]
                            )
                        TkT.append(tkt3)
                        # stage B: cheb matmul -> hps[b][half] [FOUT, 512] psum
                        hps = [[None, None], [None, None]]
                        for b in range(2):
                            for h in range(mts * P // 512):
                                ps = p_psM.tile(
                                    [P, 512], f32, name="hps", tag="hps",
                                    space="PSUM",
                                )
                                hps[b][h] = ps
                                for k in range(K):
                                    nc.tensor.matmul(
                                        ps[:],
                                        cw_sb[k][b][:],
                                        TkT[k][:, h * 512 : (h + 1) * 512],
                                        start=(k == 0),
                                        stop=(k == K - 1),
                                    )
                        # stage C: relu+LN1 per [128] target tile; build h1T
                        h1T = []
                        for b in range(2):
                            h1t = p_h1.tile(
                                [P, mts * P], f32, name=f"h1T{b}", tag=f"h1T{b}"
                            )
                            hTs_l = []
                            for h in range(mts * P // 512):
                                hTs = p_cs.tile([P, 512], f32, name="hTs", tag="hTs")
                                nc.scalar.activation(
                                    hTs[:],
                                    hps[b][h][:],
                                    Act.Relu,
                                    bias=cb_sb[:, :1],
                                    scale=1.0,
                                )
                                hTs_l.append(hTs)
                            for t0 in range(0, mts, 4):
                                psb4 = p_psT.tile(
                                    [P, 4 * P], f32, name="psb4", tag="pst",
                                    space="PSUM",
                                )
                                for i in range(4):
                                    t = t0 + i
                                    h, off = divmod(t * P, 512)
                                    psr = p_psT.tile(
                                        [P, P], f32, name="psr", tag="pst",
                                        space="PSUM",
                                    )
                                    nc.tensor.transpose(
                                        psr[:], hTs_l[h][:, off : off + P], ident[:]
                                    )
                                    h1row = p_cs.tile(
                                        [P, P], f32, name="h1row", tag="h1row"
                                    )
                                    emit_ln(psr[:], h1row[:], ln1_gb, p_small, p_cs)
                                    nc.tensor.transpose(
                                        psb4[:, i * P : (i + 1) * P], h1row[:],
                                        ident[:],
                                    )
                                nc.vector.tensor_copy(
                                    out=h1t[:, t0 * P : (t0 + 4) * P], in_=psb4[:]
                                )
                            h1T.append(h1t)
                        # stage D: pseudo-conv matmul -> h2ps[b] [FOUT, mts*32]
                        noc = mts * P // 4  # out targets per supertile
                        h2ps = []
                        for b in range(2):
                            ps = p_psM.tile(
                                [P, noc], f32, name="h2ps", tag="hps", space="PSUM"
                            )
                            rview = h1T[b][:].rearrange("p (i r) -> p r i", r=4)
                            for r in range(4):
                                nc.tensor.matmul(
                                    ps[:],
                                    pw_sb[r][:],
                                    rview[:, r, :],
                                    start=(r == 0),
                                    stop=(r == 3),
                                )
                            h2ps.append(ps)
                        # stage E: relu+LN2, write out
                        for b in range(2):
                            h2Ts = p_cs.tile([P, noc], f32, name="h2Ts", tag="hTs")
                            nc.scalar.activation(
                                h2Ts[:],
                                h2ps[b][:],
                                Act.Relu,
                                bias=pb_sb[:, :1],
                                scale=1.0,
                            )
                            for j in range(noc // P):
                                psr2 = p_psT.tile(
                                    [P, P], f32, name="psr2", tag="pst",
                                    space="PSUM",
                                )
                                nc.tensor.transpose(
                                    psr2[:], h2Ts[:, j * P : (j + 1) * P], ident[:]
                                )
                                orow = p_cs.tile([P, P], f32, name="orow", tag="orow")
                                emit_ln(psr2[:], orow[:], ln2_gb, p_small, p_cs)
                                o0 = m * (mts * P // 4) + j * P
                                nc.sync.dma_start(
                                    out=out_d.ap()[b, o0 : o0 + P, :], in_=orow[:]
                                )

    nc.compile()
    return nc


# ------------------------------------------------------------------ entry


@functools.lru_cache(maxsize=4)
def _compiled(cfg: Cfg):
    return build_nc(cfg)


def kernel(**inputs) -> np.ndarray:
    from concourse.bass_utils import run_bass_kernel_spmd

    n = inputs["x"].shape[1]
    cfg = dataclasses.replace(
        FULL,
        n=n,
        l1aff=not (
            np.all(np.asarray(inputs["gamma1"]) == 1.0)
            and np.all(np.asarray(inputs["beta1"]) == 0.0)
        ),
        l2aff=not (
            np.all(np.asarray(inputs["gamma2"]) == 1.0)
            and np.all(np.asarray(inputs["beta2"]) == 0.0)
        ),
    )
    nc = _compiled(cfg)
    in_maps = host_prep(inputs, cfg)
    res = run_bass_kernel_spmd(nc, in_maps, list(range(cfg.ncores)))
    out = np.concatenate([res.results[i]["out"] for i in range(cfg.ncores)], axis=1)
    return np.ascontiguousarray(out)
